# revision 1
# baseline (speedup 1.0000x reference)
"""Attention-LSTM captioning model, data-parallel over batch on 8 NeuronCores.

Contract: kernel(**inputs) takes FULL (unsharded) numpy inputs keyed as in
setup_inputs() and returns the FULL [B, T-1, V] float32 output.

Sharding: batch 64 -> 8 cores x 8 rows (hint: data-parallel over batch,
weights replicated). The embedding gather (emb[seq]) is done host-side (it is
pure indexing); everything else runs on the NeuronCores via a single jitted
shard_map program (PJRT/axon path - the same execution route
bass_utils.run_bass_kernel_spmd uses in this environment).
"""

import numpy as np
import jax
import jax.numpy as jnp
from jax.sharding import Mesh, PartitionSpec as P
from jax.experimental.shard_map import shard_map

N_CORES = 8
H = 512
F = 2048
V = 9488
L = 196
T = 17  # seq length; T-1 = 16 recurrent steps

_CACHE = {}


def _mm_bf16(a, b):
    # bf16 operands, fp32 accumulate: PE runs bf16 at 1 cycle/row vs fp32's 4.
    return jnp.matmul(a.astype(jnp.bfloat16), b.astype(jnp.bfloat16),
                      preferred_element_type=jnp.float32)


def _model(fc, att, xts, lin_W, lin_b, Wih, Whh, ctx_W, ctx_b,
           h2a_W, h2a_b, alpha_W, alpha_b, logit_W, logit_b):
    # Per-core shapes: fc [b,F], att [b,L,F], xts [b,T-1,H]; weights replicated.
    h = fc @ lin_W.T + lin_b                      # [b,H]
    c = h
    p_att = jnp.einsum('blf,hf->blh', att.astype(jnp.bfloat16),
                       ctx_W.astype(jnp.bfloat16),
                       preferred_element_type=jnp.float32) + ctx_b  # [b,L,H]
    outs = []
    for t in range(T - 1):
        xt = xts[:, t, :]                          # [b,H]
        att_h = h @ h2a_W.T + h2a_b                # [b,H]
        dot = jnp.tanh(p_att + att_h[:, None, :])  # [b,L,H]
        e = jnp.einsum('blh,h->bl', dot, alpha_W[0]) + alpha_b[0]
        w = jax.nn.softmax(e, axis=-1)             # [b,L]
        att_res = jnp.einsum('bl,blf->bf', w, att)  # [b,F]
        x = jnp.concatenate([xt, att_res], axis=1)  # [b,H+F]
        gates = _mm_bf16(x, Wih.T) + _mm_bf16(h, Whh.T)  # [b,4H]
        i_g = gates[:, 0 * H:1 * H]
        f_g = gates[:, 1 * H:2 * H]
        g_g = gates[:, 2 * H:3 * H]
        o_g = gates[:, 3 * H:4 * H]
        c = jax.nn.sigmoid(f_g) * c + jax.nn.sigmoid(i_g) * jnp.tanh(g_g)
        h = jax.nn.sigmoid(o_g) * jnp.tanh(c)
        logits = _mm_bf16(h, logit_W.T) + logit_b  # [b,V]
        logp = jax.nn.log_softmax(logits, axis=-1)
        outs.append(logp)
    return jnp.stack(outs, axis=1)                 # [b,T-1,V]


def get_compiled():
    """Jitted SPMD function over the 8 NeuronCores (cached)."""
    if 'fn' in _CACHE:
        return _CACHE['fn'], _CACHE['mesh']
    devs = jax.devices()[:N_CORES]
    assert len(devs) == N_CORES, f"need {N_CORES} devices, have {jax.devices()}"
    mesh = Mesh(np.asarray(devs), ('core',))
    sharded = (P('core'), P('core'), P('core'))
    repl = tuple(P() for _ in range(12))
    fn = jax.jit(shard_map(
        _model, mesh=mesh,
        in_specs=sharded + repl,
        out_specs=P('core'),
        check_rep=False,
    ))
    _CACHE['fn'] = fn
    _CACHE['mesh'] = mesh
    return fn, mesh


def prepare_args(fc_feats, att_feats, seq, lin_W, lin_b, emb, Wih, Whh,
                 ctx_W, ctx_b, h2a_W, h2a_b, alpha_W, alpha_b,
                 logit_W, logit_b):
    """Host-side preprocessing: embedding gather + dtype normalization."""
    f32 = np.float32
    seq = np.asarray(seq)
    emb_np = np.asarray(emb, f32)
    xts = emb_np[seq[:, :-1]]                      # [B,T-1,H] host gather
    args = (
        np.asarray(fc_feats, f32),
        np.asarray(att_feats, f32),
        np.ascontiguousarray(xts, f32),
        np.asarray(lin_W, f32), np.asarray(lin_b, f32),
        np.asarray(Wih, f32), np.asarray(Whh, f32),
        np.asarray(ctx_W, f32), np.asarray(ctx_b, f32),
        np.asarray(h2a_W, f32), np.asarray(h2a_b, f32),
        np.asarray(alpha_W, f32), np.asarray(alpha_b, f32),
        np.asarray(logit_W, f32), np.asarray(logit_b, f32),
    )
    return args


def kernel(fc_feats, att_feats, seq, lin_W, lin_b, emb, Wih, Whh,
           ctx_W, ctx_b, h2a_W, h2a_b, alpha_W, alpha_b,
           logit_W, logit_b):
    args = prepare_args(fc_feats, att_feats, seq, lin_W, lin_b, emb, Wih, Whh,
                        ctx_W, ctx_b, h2a_W, h2a_b, alpha_W, alpha_b,
                        logit_W, logit_b)
    fn, _ = get_compiled()
    out = fn(*args)
    return np.asarray(jax.block_until_ready(out), np.float32)

